# revision 12
# baseline (speedup 1.0000x reference)
"""PointNet-style encoder (conv1x1 stack + ragged segment-max) on 8 Trainium2 cores.

Strategy
--------
* BN folded into the conv weights host-side; every layer becomes matmul+bias+ReLU.
* Feature-major on device: activations live as [C, points] tiles, points stream
  through the PE as the matmul free dimension in 512-point macro-tiles.
* Dense point packing: segments are NOT padded to 512-point multiples. Each
  core's ceil(B/8)=4 segments are concatenated with only <=127-point alignment
  pads (duplicated own points; exact under max-pooling). Segment boundaries are
  confined to a few shared static WINDOW tiles; window tiles are processed at
  128-column quarter granularity (static program structure), and which quarter
  belongs to which segment is per-core mask DATA. All other tiles are single
  whole pieces. This takes the per-core column count from 35*512 (whole-tile
  padding) to ~32.75*512 while keeping one SPMD program.
* The mid-network segment-max feeds back via the concat identity
  concat(f2, g) @ W3 = f2 @ W3[:256] + g @ W3[256:]; the g-contribution plus
  b3 becomes a per-piece bias column (table U), applied by the activation
  instruction's per-partition bias operand.
* Phase A (layers 1-2, storing f2 in fp16 + per-piece maxes) and phase B
  (layers 3-4) are emitted as ONE interleaved pipeline: host-computed static
  bounds let per-slot g reductions and chunked U-table production run
  mid-stream, so phase A's DVE/ACT work hides under phase B's PE-bound matmuls.
* Per-piece layer-4 maxes are returned raw (streamed out mid-kernel in two DMA
  chunks); the host applies relu(. + b4) and the per-segment max over pieces
  (monotonicity makes this exact).
* Matmuls run in float16 (fp32 PSUM accumulate): 1 cycle/column, ~1e-3 rel err.
"""

import numpy as np
from itertools import permutations

import concourse.bass as bass
import concourse.mybir as mybir
import concourse.tile as tile
from concourse import bacc
from concourse.bass_utils import run_bass_kernel_spmd

N_CORES = 8
PT = 512  # points per macro-tile
QT = 128  # quarter width inside window tiles
CH = 4  # U-table production chunk (pieces)
EPS = 1e-3  # keras BatchNormalization default epsilon

F32 = mybir.dt.float32
F16 = mybir.dt.float16
AF = mybir.ActivationFunctionType
AXX = mybir.AxisListType.X
ALU_MAX = mybir.AluOpType.max

_PROGRAM_CACHE: dict = {}


def _layout(T_cols: int, wtiles: tuple):
    """Static piece layout: window tiles split into 128-col quarters."""
    T = (T_cols + PT - 1) // PT
    lens = [min(PT, T_cols - t * PT) for t in range(T)]
    pieces = []  # (tile, a, b)
    tile_pieces = [[] for _ in range(T)]
    for t in range(T):
        if t in wtiles:
            qs = [(t, a, min(a + QT, lens[t])) for a in range(0, lens[t], QT)]
        else:
            qs = [(t, 0, lens[t])]
        for q in qs:
            tile_pieces[t].append(len(pieces))
            pieces.append(q)
    return T, lens, pieces, tile_pieces


def _build_program(T_cols: int, wtiles: tuple, S: int, bounds: tuple, s_his: tuple):
    """One SPMD program for all cores.

    bounds[s]: phase-A tile count after which slot s is complete on every core.
    s_his[k]: max slot id appearing in piece chunk k (size CH) on any core.
    """
    T, lens, pieces, tile_pieces = _layout(T_cols, wtiles)
    NP = len(pieces)
    NPp = NP + (NP % 2)
    nchunks = (NP + CH - 1) // CH
    # piece-column counts completed after `bounds[s]` phase-A tiles
    pbounds = [max(tile_pieces[bounds[s] - 1]) + 1 for s in range(S)]
    uchunk_of_tile = [max(tile_pieces[t]) // CH for t in range(T)]

    nc = bacc.Bacc("TRN2")

    xT = nc.dram_tensor("xT", [3, T_cols], F16, kind="ExternalInput")
    mask = nc.dram_tensor("mask", [128, S, NPp], F32, kind="ExternalInput")
    w1 = nc.dram_tensor("w1", [3, 128], F16, kind="ExternalInput")
    w2 = nc.dram_tensor("w2", [128, 2, 128], F16, kind="ExternalInput")
    w3a = nc.dram_tensor("w3a", [128, 2, 4, 128], F16, kind="ExternalInput")
    w3b = nc.dram_tensor("w3b", [128, 2, 4, 128], F16, kind="ExternalInput")
    w4 = nc.dram_tensor("w4", [128, 4, 8, 128], F16, kind="ExternalInput")
    b1 = nc.dram_tensor("b1", [128, 1], F32, kind="ExternalInput")
    b2 = nc.dram_tensor("b2", [128, 2], F32, kind="ExternalInput")
    b3 = nc.dram_tensor("b3", [128, 4], F32, kind="ExternalInput")
    mx4 = nc.dram_tensor("mx4", [128, 8, NP], F32, kind="ExternalOutput")

    with tile.TileContext(nc) as tc:
        with (
            tc.tile_pool(name="const", bufs=1) as constp,
            tc.tile_pool(name="xp", bufs=4) as xp,
            tc.tile_pool(name="h1p", bufs=3) as h1p,
            tc.tile_pool(name="h3p", bufs=3) as h3p,
            tc.tile_pool(name="tmpp", bufs=4) as tmpp,
            tc.tile_pool(name="psA", bufs=2, space="PSUM") as psA,
            tc.tile_pool(name="psB3", bufs=2, space="PSUM") as psB3,
            tc.tile_pool(name="psB4", bufs=2, space="PSUM") as psB4,
        ):
            xTr0 = xT.ap()
            # prefetch the first x tiles ahead of the constant DMAs so the
            # first L1 matmul isn't queued behind them
            x_pre = {}
            for t0 in range(min(4, T)):
                x_sb0 = xp.tile([3, PT], F16, tag="x", name=f"x_{t0}")
                nc.sync.dma_start(out=x_sb0[:, : lens[t0]], in_=xTr0[:, t0 * PT : t0 * PT + lens[t0]])
                x_pre[t0] = x_sb0

            # small, immediately-needed constants on the sync DMA queue; the
            # big phase-B weights on the gpsimd queue so they don't
            # head-of-line-block phase A's x-tile loads.
            w1_sb = constp.tile([3, 128], F16)
            nc.sync.dma_start(out=w1_sb, in_=w1.ap())
            w2_sb = constp.tile([128, 2, 128], F16)
            nc.sync.dma_start(out=w2_sb, in_=w2.ap())
            b1_sb = constp.tile([128, 1], F32)
            nc.sync.dma_start(out=b1_sb, in_=b1.ap())
            b2_sb = constp.tile([128, 2], F32)
            nc.sync.dma_start(out=b2_sb, in_=b2.ap())
            b3_sb = constp.tile([128, 4], F32)
            nc.sync.dma_start(out=b3_sb, in_=b3.ap())
            mask_sb = constp.tile([128, S, NPp], F32)
            nc.sync.dma_start(out=mask_sb, in_=mask.ap())
            w3a_sb = constp.tile([128, 2, 4, 128], F16)
            nc.gpsimd.dma_start(out=w3a_sb, in_=w3a.ap())
            w3b_sb = constp.tile([128, 2, 4, 128], F16)
            nc.gpsimd.dma_start(out=w3b_sb, in_=w3b.ap())
            w4_sb = constp.tile([128, 4, 8, 128], F16)
            nc.gpsimd.dma_start(out=w4_sb, in_=w4.ap())

            NQ = PT // QT
            f2_all = constp.tile([128, T, 2, NQ, QT], F16)
            Mx2_sb = constp.tile([128, 2, NP], F32)
            g_sb = constp.tile([128, 2, S], F32)
            Gacc_sb = constp.tile([128, 2, NPp], F32)
            G2_sb = constp.tile([128, 2, NPp], F16)
            U_sb = constp.tile([128, 4, NPp], F32)
            Mx4_sb = constp.tile([128, 8, NP], F32)

            xTr = xT.ap()

            # HAM warmup: dependency-free matmuls keep the PE busy through the
            # DMA prologue so the clock gate opens (1.2 -> 2.4 GHz) before the
            # real matmuls start.
            warm_src = constp.tile([128, PT], F16, name="warm_src")
            nc.vector.memset(warm_src, 0.01)
            warm_out = constp.tile([128, 1], F32, name="warm_out")
            # touch the scalar engine early so its ACT_TABLE_LOAD happens
            # during the DMA prologue instead of gating the first real Relu
            warm_act = constp.tile([128, 8], F32, name="warm_act")
            nc.scalar.activation(out=warm_act, in_=warm_src[:, 0:8], func=AF.Relu)
            ps_w = psA.tile([128, PT], F32, tag="psa", name="ps_warm")
            for i in range(20):
                nc.tensor.matmul(
                    ps_w[:, :], warm_src[:, 0:128], warm_src[:, :], start=True, stop=True
                )
            nc.vector.tensor_reduce(out=warm_out, in_=ps_w[:, 0:8], axis=AXX, op=ALU_MAX)

            # ---------------- emission helpers ----------------
            deferred_reduce: list = []

            def emit_mx2(t):
                ps = tile_pieces[t]
                nq = lens[t] // QT
                if len(ps) == 1:
                    # whole-tile piece: one grouped reduce over both free dims
                    nc.vector.tensor_reduce(
                        out=Mx2_sb[:, :, ps[0] : ps[0] + 1], in_=f2_all[:, t, :, :nq, :],
                        axis=mybir.AxisListType.XY, op=ALU_MAX,
                    )
                else:
                    # window tile: one reduce yields all quarter columns at once
                    nc.vector.tensor_reduce(
                        out=Mx2_sb[:, :, ps[0] : ps[0] + nq], in_=f2_all[:, t, :, :nq, :],
                        axis=AXX, op=ALU_MAX,
                    )

            def emit_A(t, defer_reduce=False):
                """L1+L2 for tile t; stores f2 (fp16) and its per-piece maxes."""
                L = lens[t]
                if t in x_pre:
                    x_sb = x_pre.pop(t)
                else:
                    x_sb = xp.tile([3, PT], F16, tag="x", name=f"x_{t}")
                    nc.sync.dma_start(out=x_sb[:, :L], in_=xTr[:, t * PT : t * PT + L])
                ps1 = psA.tile([128, PT], F32, tag="psa", name=f"ps1_{t}")
                nc.tensor.matmul(ps1[:, :L], w1_sb[:, :], x_sb[:, :L], start=True, stop=True)
                h1_sb = h1p.tile([128, PT], F16, tag="h1", name=f"h1_{t}")
                nc.scalar.activation(out=h1_sb[:, :L], in_=ps1[:, :L], func=AF.Relu, bias=b1_sb[:, 0:1])
                for c in range(2):
                    ps2 = psA.tile([128, PT], F32, tag="psa", name=f"ps2_{t}_{c}")
                    nc.tensor.matmul(ps2[:, :L], w2_sb[:, c, :], h1_sb[:, :L], start=True, stop=True)
                    if c == 0 and len(tile_pieces[t]) == 1:
                        nc.scalar.activation(
                            out=f2_all[:, t, c, : L // QT, :], in_=ps2[:, :L], func=AF.Relu, bias=b2_sb[:, c : c + 1]
                        )
                    else:
                        # relu(x + b) on the DVE to balance ACT/DVE load
                        nc.vector.tensor_scalar(
                            f2_all[:, t, c, : L // QT, :], ps2[:, :L], b2_sb[:, c : c + 1], 0.0,
                            mybir.AluOpType.add, ALU_MAX,
                        )
                if defer_reduce:
                    deferred_reduce.append(t)
                else:
                    emit_mx2(t)

            def emit_g(s):
                """Per-slot max over the (host-bounded) range of Mx2 columns."""
                pb = pbounds[s]
                for c in range(2):
                    tmp = tmpp.tile([128, NPp], F32, tag="tmp", name=f"tmpg_{c}_{s}")
                    nc.vector.tensor_mul(tmp[:, :pb], Mx2_sb[:, c, :pb], mask_sb[:, s, :pb])
                    nc.vector.tensor_reduce(
                        out=g_sb[:, c, s : s + 1], in_=tmp[:, :pb], axis=AXX, op=ALU_MAX
                    )

            def emit_Uchunk(k):
                """U[:, :, kCH:kCH+w] = W3b.T @ G2_chunk + b3 (per-piece bias)."""
                c0 = k * CH
                w = min(CH, NP - c0)
                we = w + (w % 2)  # keep matmul free dims even
                shi = s_his[k]
                for c in range(2):
                    nc.vector.tensor_scalar_mul(
                        Gacc_sb[:, c, c0 : c0 + we], mask_sb[:, 0, c0 : c0 + we], g_sb[:, c, 0:1]
                    )
                    for s in range(1, shi + 1):
                        tmp2 = tmpp.tile([128, CH + 1], F32, tag="tmp2", name=f"tmpe_{k}_{c}_{s}")
                        nc.vector.tensor_scalar_mul(
                            tmp2[:, :we], mask_sb[:, s, c0 : c0 + we], g_sb[:, c, s : s + 1]
                        )
                        nc.vector.tensor_add(
                            Gacc_sb[:, c, c0 : c0 + we], Gacc_sb[:, c, c0 : c0 + we], tmp2[:, :we]
                        )
                    nc.scalar.copy(G2_sb[:, c, c0 : c0 + we], Gacc_sb[:, c, c0 : c0 + we])
                for m in range(4):
                    psu = psA.tile([128, PT], F32, tag="psa", name=f"psu_{k}_{m}")
                    nc.tensor.matmul(
                        psu[:, :we], w3b_sb[:, 0, m, :], G2_sb[:, 0, c0 : c0 + we],
                        start=True, stop=False,
                    )
                    nc.tensor.matmul(
                        psu[:, :we], w3b_sb[:, 1, m, :], G2_sb[:, 1, c0 : c0 + we],
                        start=False, stop=True,
                    )
                    nc.scalar.activation(
                        out=U_sb[:, m, c0 : c0 + we], in_=psu[:, :we],
                        func=AF.Identity, bias=b3_sb[:, m : m + 1],
                    )

            h3_tiles = {}

            def emit_L3(t):
                L = lens[t]
                h3_sb = h3p.tile([128, 4, PT], F16, tag="h3", name=f"h3_{t}")
                for m in range(4):
                    ps3 = psB3.tile([128, PT], F32, tag="ps3", name=f"ps3_{t}_{m}")
                    nc.tensor.matmul(
                        ps3[:, :L], w3a_sb[:, 0, m, :], f2_all[:, t, 0, : L // QT, :], start=True, stop=False
                    )
                    nc.tensor.matmul(
                        ps3[:, :L], w3a_sb[:, 1, m, :], f2_all[:, t, 1, : L // QT, :], start=False, stop=True
                    )
                    for p in tile_pieces[t]:
                        _, a, b = pieces[p]
                        nc.scalar.activation(
                            out=h3_sb[:, m, a:b], in_=ps3[:, a:b], func=AF.Relu,
                            bias=U_sb[:, m, p : p + 1],
                        )
                h3_tiles[t] = h3_sb

            def emit_L4(t):
                L = lens[t]
                nq = L // QT
                ps = tile_pieces[t]
                h3_sb = h3_tiles.pop(t)
                for mg in range(4):
                    # inner dims pad to a full PSUM bank (512 f32) so each
                    # m-chunk's matmul output stays within one bank
                    ps4 = psB4.tile([128, 2, NQ, QT], F32, tag="ps4", name=f"ps4_{t}_{mg}")
                    for mi in range(2):
                        m = mg * 2 + mi
                        for k in range(4):
                            nc.tensor.matmul(
                                ps4[:, mi, :nq, :], w4_sb[:, k, m, :], h3_sb[:, k, :L],
                                start=(k == 0), stop=(k == 3),
                            )
                    if len(ps) == 1:
                        nc.vector.tensor_reduce(
                            out=Mx4_sb[:, 2 * mg : 2 * mg + 2, ps[0] : ps[0] + 1],
                            in_=ps4[:, :, :nq, :], axis=mybir.AxisListType.XY, op=ALU_MAX,
                        )
                    else:
                        nc.vector.tensor_reduce(
                            out=Mx4_sb[:, 2 * mg : 2 * mg + 2, ps[0] : ps[0] + nq],
                            in_=ps4[:, :, :nq, :], axis=AXX, op=ALU_MAX,
                        )

            # ---------------- interleaved pipeline ----------------
            a_next = 0
            b_next = 0
            l3_next = 0
            u_next = 0
            g_emitted = [False] * S
            mx4_sent = 0

            def try_unlock():
                nonlocal u_next
                for s in range(S):
                    if not g_emitted[s] and a_next >= bounds[s]:
                        # flush reduces this slot's g depends on
                        for t in [d for d in deferred_reduce if d < bounds[s]]:
                            emit_mx2(t)
                            deferred_reduce.remove(t)
                        emit_g(s)
                        g_emitted[s] = True
                while u_next < nchunks and all(g_emitted[s] for s in range(s_his[u_next] + 1)):
                    emit_Uchunk(u_next)
                    u_next += 1

            # phase A must lead phase B by enough tiles that B's U-table
            # chunks are always unlocked when its L3s reach the PE queue
            need = [bounds[s_his[uchunk_of_tile[min(i + 1, T - 1)]]] for i in range(T)]
            LEAD = max(max(need[i] - i for i in range(T)) + 1, need[0])

            # the fill phase is DVE/ACT-paced with the PE at ~50% duty, which
            # re-throttles the clock gate; pad it with dummy matmuls into a
            # psB4-pool tile (idle until the first L4, released before the
            # second one needs its slot)
            ps_dummy = psB4.tile([128, 2, NQ, QT], F32, tag="ps4", name="ps_dummy")

            while b_next < T:
                while a_next < min(T, b_next + LEAD):
                    # fill-tail tiles (beyond slot 0 on every core) defer their
                    # DVE reduce into the stream's slack
                    emit_A(a_next, defer_reduce=(b_next == 0 and a_next >= bounds[0]))
                    if b_next == 0:
                        for _ in range(4):
                            nc.tensor.matmul(
                                ps_dummy[:, 0, :, :], warm_src[:, 0:128], warm_src[:, :],
                                start=True, stop=True,
                            )
                    a_next += 1
                    try_unlock()
                progressed = False
                while (
                    l3_next <= min(b_next + 1, T - 1)
                    and uchunk_of_tile[l3_next] < u_next
                    and l3_next < a_next
                ):
                    emit_L3(l3_next)
                    l3_next += 1
                    progressed = True
                if l3_next > b_next:
                    if b_next == 0:
                        # bridge the prologue stall (L4(0) waiting on the first
                        # h3 activations) so the clock gate stays open
                        for _ in range(26):
                            nc.tensor.matmul(
                                ps_dummy[:, 0, :, :], warm_src[:, 0:128], warm_src[:, :],
                                start=True, stop=True,
                            )
                    emit_L4(b_next)
                    b_next += 1
                    progressed = True
                    # stream completed Mx4 columns out mid-kernel so the final
                    # DMA doesn't serialize behind the last tile
                    if b_next == (2 * T) // 3:
                        pdone = max(tile_pieces[b_next - 1]) + 1
                        nc.gpsimd.dma_start(
                            out=mx4.ap()[:, :, :pdone], in_=Mx4_sb[:, :, :pdone]
                        )
                        mx4_sent = pdone
                if not progressed:
                    if a_next < T:
                        emit_A(a_next)
                        a_next += 1
                        try_unlock()
                    else:
                        raise RuntimeError("pipeline deadlock")

            nc.sync.dma_start(out=mx4.ap()[:, :, mx4_sent:], in_=Mx4_sb[:, :, mx4_sent:])

    nc.finalize()
    return nc


def _a128(v):
    return ((int(v) + 127) // 128) * 128


def _prepare(x, seg_ids, B):
    """Dense-pack segments; boundaries 128-aligned inside shared window tiles."""
    counts = np.bincount(seg_ids, minlength=B)
    starts = np.concatenate([[0], np.cumsum(counts)])
    npts = counts.astype(int)
    S = (B + N_CORES - 1) // N_CORES

    # assign segments to cores (S per core), balancing total points
    order = np.argsort(-npts, kind="stable")
    assign: list[list[int]] = [[] for _ in range(N_CORES)]
    loads = [0] * N_CORES
    for s in order:
        cands = [c for c in range(N_CORES) if len(assign[c]) < S]
        c = min(cands, key=lambda i: loads[i])
        assign[c].append(int(s))
        loads[c] += int(npts[s])
    for _ in range(5000):
        hi = max(range(N_CORES), key=lambda i: loads[i])
        improved = False
        for lo in sorted(range(N_CORES), key=lambda i: loads[i]):
            if lo == hi:
                continue
            for ia, sa in enumerate(assign[hi]):
                for ib, sb in enumerate(assign[lo]):
                    d = npts[sa] - npts[sb]
                    if d > 0 and max(loads[hi] - d, loads[lo] + d) < max(loads[hi], loads[lo]):
                        assign[hi][ia], assign[lo][ib] = sb, sa
                        loads[hi] -= d
                        loads[lo] += d
                        improved = True
                        break
                if improved:
                    break
            if improved:
                break
        if not improved:
            break

    # per-core slot order: coordinate descent minimizing (T_cols, #window tiles)
    def plan(orders, LB=None):
        LB = LB or [0] * (S - 1)
        cur = np.zeros(N_CORES, dtype=int)
        wins = []
        for k in range(S - 1):
            nat = cur + np.array([npts[orders[c][k]] for c in range(N_CORES)])
            ali = np.maximum(np.array([_a128(v) for v in nat]), LB[k])
            wins.append((int(ali.min() // PT), int((ali.max() - 1) // PT)))
            cur = ali
        tot = cur + np.array([npts[orders[c][S - 1]] for c in range(N_CORES)])
        T_cols = _a128(int(tot.max()))
        nwin = sum(hi - lo + 1 for lo, hi in wins)
        return wins, T_cols, cur, tot, nwin

    orders = [sorted(assign[c], key=lambda s: -npts[s]) for c in range(N_CORES)]
    for _sweep in range(6):
        changed = False
        for c in range(N_CORES):
            bl, bo = None, None
            for perm in permutations(assign[c]):
                test = [list(o) for o in orders]
                test[c] = list(perm)
                _, T_cols, _, tot, nwin = plan(test)
                key = (T_cols, nwin, int(tot.max()))
                if bl is None or key < bl:
                    bl, bo = key, list(perm)
            if bo != orders[c]:
                changed = True
            orders[c] = bo
        if not changed:
            break

    # shrink windows from the left wherever lifting the low cores costs no
    # extra tiles (pads on non-critical cores are free)
    LB = [0] * (S - 1)
    base_T = plan(orders, LB)[1]
    for k in range(S - 2, -1, -1):
        while True:
            wins, T_cols, _, _, _ = plan(orders, LB)
            lo, hi = wins[k]
            if lo >= hi:
                break
            trial = list(LB)
            trial[k] = PT * (lo + 1)
            if plan(orders, trial)[1] <= base_T:
                LB = trial
            else:
                break
    wins, T_cols, _, _, _ = plan(orders, LB)
    wtiles = tuple(sorted(set(t for lo, hi in wins for t in range(lo, hi + 1))))

    T, lens, pieces, tile_pieces = _layout(T_cols, wtiles)
    NP = len(pieces)
    NPp = NP + (NP % 2)

    # per-core packed x and piece->slot masks
    xT_cores, mask_cores, post = [], [], []
    for c in range(N_CORES):
        parts, cutcols = [], []
        cum = 0
        for s in range(S):
            seg = orders[c][s]
            pts = x[starts[seg] : starts[seg + 1]]
            width = (max(_a128(cum + len(pts)), LB[s]) if s < S - 1 else T_cols) - cum
            reps = (width + len(pts) - 1) // len(pts)
            if reps > 1:
                pts = np.concatenate([pts] * reps)
            parts.append(pts[:width])
            cum += width
            cutcols.append(cum)
        xc = np.concatenate(parts).astype(np.float16)
        xT_cores.append(np.ascontiguousarray(xc.T))
        # piece p (cols [ga, gb)) belongs to slot = #cuts <= ga
        pslot = np.empty(NP, dtype=int)
        for p, (t, a, b) in enumerate(pieces):
            ga = t * PT + a
            pslot[p] = sum(1 for cc in cutcols[:-1] if cc <= ga)
        m01 = np.zeros((S, NPp), np.float32)
        m01[pslot, np.arange(NP)] = 1.0
        mask_cores.append(np.ascontiguousarray(np.broadcast_to(m01[None], (128, S, NPp))))
        post.append((orders[c], pslot))

    # shared pipeline bounds: slot s complete after its last piece's tile (max
    # over cores); with per-core pslot, use the mask to find the last piece.
    bounds = []
    for s in range(S):
        last_tile = 0
        for c in range(N_CORES):
            ps = np.flatnonzero(post[c][1] == s)
            if len(ps):
                last_tile = max(last_tile, pieces[ps.max()][0])
        bounds.append(last_tile + 1)
    nchunks = (NP + CH - 1) // CH
    s_his = []
    for k in range(nchunks):
        hi = 0
        for c in range(N_CORES):
            hi = max(hi, int(post[c][1][k * CH : min((k + 1) * CH, NP)].max()))
        s_his.append(hi)
    return T_cols, wtiles, S, tuple(bounds), tuple(s_his), xT_cores, mask_cores, post


def make_in_maps(inputs):
    """Fold BN, pack points, and build the per-core SPMD input dicts.

    Returns (key, in_maps, post, b4f) where key indexes _PROGRAM_CACHE.
    """
    x = np.asarray(inputs["x"], np.float32)
    seg_ids = np.asarray(inputs["seg_ids"])
    B = int(inputs["num_segments"])

    Wf, bf = [], []
    for i in (1, 2, 3, 4):
        W = np.asarray(inputs[f"W{i}"], np.float32)
        b = np.asarray(inputs[f"b{i}"], np.float32)
        ga = np.asarray(inputs[f"g{i}"], np.float32)
        be = np.asarray(inputs[f"be{i}"], np.float32)
        m = np.asarray(inputs[f"m{i}"], np.float32)
        v = np.asarray(inputs[f"v{i}"], np.float32)
        sc = ga / np.sqrt(v + EPS)
        Wf.append(np.ascontiguousarray(W * sc[None, :]))
        bf.append((b - m) * sc + be)
    W1f, W2f, W3f, W4f = Wf
    b1f, b2f, b3f, b4f = bf

    T_cols, wtiles, S, bounds, s_his, xT_cores, mask_cores, post = _prepare(x, seg_ids, B)

    w1d = W1f.astype(np.float16)
    w2d = np.ascontiguousarray(W2f.reshape(128, 2, 128).astype(np.float16))
    w3ad = np.ascontiguousarray(W3f[:256].reshape(2, 128, 4, 128).transpose(1, 0, 2, 3).astype(np.float16))
    w3bd = np.ascontiguousarray(W3f[256:].reshape(2, 128, 4, 128).transpose(1, 0, 2, 3).astype(np.float16))
    w4d = np.ascontiguousarray(W4f.reshape(4, 128, 8, 128).transpose(1, 0, 2, 3).astype(np.float16))
    b1d = np.ascontiguousarray(b1f.reshape(128, 1))
    b2d = np.ascontiguousarray(b2f.reshape(2, 128).T)
    b3d = np.ascontiguousarray(b3f.reshape(4, 128).T)

    in_maps = [
        {
            "xT": xT_cores[c],
            "mask": mask_cores[c],
            "w1": w1d,
            "w2": w2d,
            "w3a": w3ad,
            "w3b": w3bd,
            "w4": w4d,
            "b1": b1d,
            "b2": b2d,
            "b3": b3d,
        }
        for c in range(N_CORES)
    ]
    return (T_cols, wtiles, S, bounds, s_his), in_maps, post, b4f


def postprocess(results, post, b4f, B):
    out = np.zeros((B, 1024), np.float32)
    for c in range(N_CORES):
        mx4 = results[c]["mx4"]  # [128, 8, NP]
        segs, pslot = post[c]
        for s, seg in enumerate(segs):
            cols = np.flatnonzero(pslot == s)
            raw = mx4[:, :, cols].max(axis=2)  # [128, 8]
            out[seg] = np.maximum(raw.T.reshape(1024) + b4f, 0.0)
    return out


def get_program(key):
    if key not in _PROGRAM_CACHE:
        _PROGRAM_CACHE[key] = _build_program(*key)
    return _PROGRAM_CACHE[key]


def kernel(**inputs) -> np.ndarray:
    B = int(inputs["num_segments"])
    key, in_maps, post, b4f = make_in_maps(inputs)
    nc = get_program(key)
    last_err = None
    for _ in range(3):  # retry transient NRT device wedges
        try:
            res = run_bass_kernel_spmd(nc, in_maps, core_ids=list(range(N_CORES)))
            return postprocess(res.results, post, b4f, B)
        except Exception as e:  # noqa: BLE001
            last_err = e
    raise last_err
